# revision 1
# baseline (speedup 1.0000x reference)
"""AggregationLoss Trainium2 kernel (nn_AggregationLoss_19258633355266).

Reference math: per sample b and instance i in 1..8, over the per-pixel
channel energy s = sum_c pred[b,c,:]^2 and instance-id maps t, k:
    ct_i = #{t==i}, ck_i = #{k==i}
    A_i  = sum s[t==i], Bk_i = sum s[k==i], D_i = sum s[(t==k)&(k==i)]
    ss   = A + Bk/ck^2 - 2 D/ck ; loss_i = log1p((sqrt(ss)-0.5)^2)/ct
summed over valid segments (ct>0, ck>0, ss>0, i>=1).

Distribution: data-parallel over batch B=16 across 8 NeuronCores
(2 samples per core). On each core the two samples are packed along the
partition axis (64 rows each), so every engine instruction covers both
samples at once and per-partition accumulators stay per-sample.

The device program is instruction-minimal (~25 instructions per core;
this environment is dominated by per-instruction overhead, not element
throughput):
  - pred is DMA'd once, squared in place on ScalarE, and channel-reduced
    with one grouped tensor_reduce -> s.
  - labels are shipped as bf16 (0..8 exact) and compared against an
    instance-index vector with broadcast access patterns: ONE
    tensor_tensor produces all 16 masks (2 maps x 8 instances),
    m16 (128, 2, 8, Wc) bf16.
  - counts  = reduce_X(m16)                       -> ct_t, ct_k
  - ms16    = m16 * s      (in-place)             -> masked energies
  - A, Bk   = reduce_X(ms16)
  - D prods = ms16[k] * (t==k) (in-place)         -> mask_k * s * mtk
  - D       = reduce_X(...)
Per-partition partial stats (128, 40) go back to the host, which does the
tiny final segment formula in float64.
"""

import sys

import numpy as np

import ml_dtypes

B = 16
C = 4
NPIX = 640 * 640
P = 128
PS = 64                    # partitions per sample
W = NPIX // PS             # 6400 free-dim elements per sample row
WC = 3200                  # mask-phase column chunk (SBUF-sized)
NCH = W // WC
B_LOC = 2                  # samples per core
N_CORES = 8
NI = 8                     # instances 1..8 (0 = background, always invalid)
NSTAT = 5 * NI
SIGMA = 0.5

_NC = None


def _import_concourse():
    try:
        import concourse.bacc  # noqa: F401
    except ImportError:
        sys.path.append("/opt/trn_rl_repo")
        import concourse.bacc  # noqa: F401


def _build_nc(repeat=1, pred_bf=False):
    _import_concourse()
    import concourse.bacc as bacc
    import concourse.mybir as mybir
    import concourse.tile as tile
    from contextlib import ExitStack

    f32 = mybir.dt.float32
    bf16 = mybir.dt.bfloat16
    pdt = bf16 if pred_bf else f32
    eq = mybir.AluOpType.is_equal
    add = mybir.AluOpType.add
    mult = mybir.AluOpType.mult
    X = mybir.AxisListType.X

    nc = bacc.Bacc("TRN2", target_bir_lowering=False, debug=False,
                   num_devices=N_CORES)
    pred_d = nc.declare_dram_parameter("pred", [B_LOC, C, PS, W], pdt, isOutput=False)
    t_d = nc.declare_dram_parameter("tlab", [B_LOC, PS, W], bf16, isOutput=False)
    k_d = nc.declare_dram_parameter("klab", [B_LOC, PS, W], bf16, isOutput=False)
    iv_d = nc.declare_dram_parameter("iv", [P, NI], bf16, isOutput=False)
    stats_d = nc.declare_dram_parameter("stats", [P, NCH * NSTAT], f32, isOutput=True)

    with tile.TileContext(nc) as tc, ExitStack() as ctx:
        cpool = ctx.enter_context(tc.tile_pool(name="c", bufs=1))
        iv = cpool.tile([P, NI], bf16)
        nc.sync.dma_start(iv[:], iv_d[:])

        for _ in range(repeat):
            L = cpool.tile([P, 2, W], bf16, tag="L")
            for b in range(B_LOC):
                nc.sync.dma_start(L[b * PS:(b + 1) * PS, 0, :], t_d[b])
                nc.sync.dma_start(L[b * PS:(b + 1) * PS, 1, :], k_d[b])
            s = cpool.tile([P, W], f32, tag="s")

            with tc.tile_pool(name="pr", bufs=1) as pr:
                predt = pr.tile([P, C, W], pdt, tag="predt")
                for b in range(B_LOC):
                    nc.sync.dma_start(
                        predt[b * PS:(b + 1) * PS, :, :],
                        pred_d[b].rearrange("c p w -> p c w"),
                    )
                nc.scalar.square(predt[:], predt[:])
                nc.vector.tensor_reduce(
                    out=s[:],
                    in_=predt[:].rearrange("p c w -> p w c"),
                    axis=X, op=add,
                )

            s_bf = cpool.tile([P, W], bf16, tag="s_bf")
            nc.vector.tensor_copy(s_bf[:], s[:])
            mtk = cpool.tile([P, W], bf16, tag="mtk")
            nc.vector.tensor_tensor(out=mtk[:], in0=L[:, 0, :], in1=L[:, 1, :], op=eq)
            stats = cpool.tile([P, NCH * NSTAT], f32, tag="stats")

            with tc.tile_pool(name="m", bufs=1) as mp:
                for ch in range(NCH):
                    c0 = ch * WC
                    sb = ch * NSTAT
                    m16 = mp.tile([P, 2, NI, WC], bf16, tag="m16")
                    nc.vector.tensor_tensor(
                        out=m16[:],
                        in0=L[:, :, c0:c0 + WC].unsqueeze(2)
                            .broadcast_to([P, 2, NI, WC]),
                        in1=iv[:].unsqueeze(1).unsqueeze(3)
                            .broadcast_to([P, 2, NI, WC]),
                        op=eq,
                    )
                    nc.vector.tensor_reduce(
                        out=stats[:, sb:sb + 16], in_=m16[:], axis=X, op=add,
                    )
                    nc.vector.tensor_tensor(
                        out=m16[:],
                        in0=m16[:],
                        in1=s_bf[:, c0:c0 + WC].unsqueeze(1).unsqueeze(1)
                            .broadcast_to([P, 2, NI, WC]),
                        op=mult,
                    )
                    nc.vector.tensor_reduce(
                        out=stats[:, sb + 16:sb + 32], in_=m16[:], axis=X, op=add,
                    )
                    nc.vector.tensor_tensor(
                        out=m16[:, 1, :, :],
                        in0=m16[:, 1, :, :],
                        in1=mtk[:, c0:c0 + WC].unsqueeze(1)
                            .broadcast_to([P, NI, WC]),
                        op=mult,
                    )
                    nc.vector.tensor_reduce(
                        out=stats[:, sb + 32:sb + 40], in_=m16[:, 1, :, :],
                        axis=X, op=add,
                    )
            nc.sync.dma_start(stats_d[:], stats[:])
    nc.finalize()
    return nc


def _get_nc():
    global _NC
    if _NC is None:
        _NC = _build_nc()
    return _NC


def _bf16(a):
    return np.asarray(a).astype(ml_dtypes.bfloat16)


def run_device(pred, tlab, klab, pred_bf=False, **spmd_kwargs):
    """Run the 8-core bass kernel; returns ((B, 40) per-sample stats, results)."""
    _import_concourse()
    from concourse.bass_utils import run_bass_kernel_spmd

    nc = _get_nc()
    if pred_bf:
        pred = _bf16(pred).reshape(B, C, PS, W)
    else:
        pred = np.ascontiguousarray(np.asarray(pred), dtype=np.float32).reshape(B, C, PS, W)
    tlab = _bf16(tlab).reshape(B, PS, W)
    klab = _bf16(klab).reshape(B, PS, W)
    iv = np.broadcast_to(
        np.arange(1, NI + 1, dtype=ml_dtypes.bfloat16), (P, NI)).copy()

    in_maps = []
    for r in range(N_CORES):
        lo, hi = r * B_LOC, (r + 1) * B_LOC
        in_maps.append({
            "pred": pred[lo:hi],
            "tlab": tlab[lo:hi],
            "klab": klab[lo:hi],
            "iv": iv,
        })

    res = run_bass_kernel_spmd(nc, in_maps, list(range(N_CORES)), **spmd_kwargs)
    stats = np.zeros((B, NSTAT), np.float64)
    for r in range(N_CORES):
        out = np.asarray(res.results[r]["stats"], dtype=np.float64)
        out = out.reshape(P, NCH, NSTAT).sum(axis=1)      # (128, 40)
        for b in range(B_LOC):
            stats[r * B_LOC + b] = out[b * PS:(b + 1) * PS].sum(axis=0)
    return stats, res


def finish(stats):
    """Final loss from per-sample stats (B, 40): [ct_t(8), ct_k(8), A(8), Bk(8), D(8)]."""
    ct = stats[:, 0:8]
    ck = stats[:, 8:16]
    A = stats[:, 16:24]
    Bk = stats[:, 24:32]
    D = stats[:, 32:40]
    kc = np.where(ck > 0, ck, 1.0)
    tcs = np.where(ct > 0, ct, 1.0)
    ss = A + Bk / (kc * kc) - 2.0 * D / kc
    ss_safe = np.where(ss > 0, ss, 1.0)
    norm = np.sqrt(ss_safe) - SIGMA
    loss = np.log1p(norm * norm) / tcs
    valid = (ct > 0) & (ck > 0) & (ss > 0)
    return np.array(np.sum(np.where(valid, loss, 0.0)), dtype=np.float32)


def kernel(pred_similarities, regions_mask=None, kernels_mask=None,
           text_mask_ndi_labels=None, kernel_mask_ndi_labels=None):
    stats, _ = run_device(pred_similarities, text_mask_ndi_labels,
                          kernel_mask_ndi_labels)
    return finish(stats)

